# revision 1
# baseline (speedup 1.0000x reference)
# Trainium2 Bass kernel for the ContextBlock problem.
#
# Reference computation (per sample b):
#   xc    = concat(x0..x3)            [C=1024, HW=4096]
#   attn  = softmax(wm @ xc)          [HW]
#   ctx   = xc @ attn                 [C]
#   mul   = residual-gated MLP stack (sigmoid branch)   [C]
#   add   = residual-gated MLP stack (linear branch)    [C]
#   out   = sum_l (x_l * mul_l + add_l)                 [CL=256, HW]
#
# Distribution: data-parallel over batch, one sample per NeuronCore (B=8).
# No collectives required.
#
# Per-core dataflow (final):
#   all weights prefetched up front (DMA overlaps pass1/pass2)
#   pass1: logits into [8,512] PSUM via zero-padded stationaries, riding
#          the x DMA stream (exp then runs on 8 partitions, not 1)
#   pass2 split across engines: slabs 0-3 are PE-transposed (after pass1,
#          when the PE is ramped) and contracted against attn columns on
#          the PE, while slabs 4-7 run scalar_tensor_tensor+accum on the
#          DVE against an attn broadcast; both halves merge into v0
#   gates: v-stationary x weight-moving matmuls (h comes out as rows; the
#          55us of per-matmul 128-row LDWEIGHTS of the weight-stationary
#          form is gone); rows return to columns via 1-col outer-product
#          matmuls; rsqrt = exp(-0.5*ln(var)) with sqrt(P)/1/P folded into
#          host-packed g and the mean broadcast; sigmoid = 1/(1+exp(-z));
#          the activation-table registry is pruned so ONE table set
#          (natural_log_exp_and_others) serves the whole kernel
#   pass3: diag(mul) @ x accumulated over levels in PSUM, bias = add-sum
#          split across DVE/scalar engines
# fp8 DoubleRow gate variants exist behind K_GATES (fp8|mix) but the
# error cost (1.3-1.9e-2 vs 4.3e-3) outweighed the ~5us gain.

import os
import numpy as np
import ml_dtypes
from contextlib import ExitStack

import concourse.bass as bass
import concourse.bacc as bacc
import concourse.mybir as mybir
import concourse.tile as tile

BF = mybir.dt.bfloat16
F8 = mybir.dt.float8e4
F32 = mybir.dt.float32
AF = mybir.ActivationFunctionType
ALU = mybir.AluOpType
AX = mybir.AxisListType
DR = mybir.MatmulPerfMode.DoubleRow

B, L, CL, H, W = 8, 4, 256, 64, 64
C = L * CL          # 1024
HW = H * W          # 4096
P = C // 4          # 256
R = 2
EPS = 1e-5
NJ = C // 128       # 8   c-slabs
NCORES = 8

PASS2 = os.environ.get("K_PASS2", "half")   # half | stt_bf
GATES = os.environ.get("K_GATES", "bf16")   # fp8 | mix | bf16
W1F8 = GATES in ("fp8", "mix")              # W1 fp8 DoubleRow
W2F8 = GATES == "fp8"                       # W2 fp8 DoubleRow

# per-round pre-LN scale of h under fp8 gate quantization (v-scale * w-scale)
KR = {0: 64.0 * 16.0, 1: 16.0 * 16.0} if W1F8 else {0: 1.0, 1: 1.0}

_CACHE = {}


def _patch_act_tables():
    """Prune our activation functions from every table set except
    natural_log_exp_and_others so the whole kernel runs on ONE set
    (no mid-kernel ACT_TABLE_LOAD switches). Dict order is preserved so
    act_func_set ids stay aligned with act_info.json."""
    if getattr(bacc, "_act_tables_patched", False):
        return
    from concourse import hw_specs
    orig = hw_specs.get_activation_tables
    mine = {AF.Exp, AF.Ln, AF.Relu, AF.Identity, AF.Copy}
    keep = "natural_log_exp_and_others"

    def patched(arch):
        tabs = orig(arch)
        out = {}
        for name, fns in tabs.items():
            out[name] = set(fns) if name == keep else set(fns) - mine
        return out

    import functools
    patched_cached = functools.cache(patched)
    bacc.get_activation_tables = patched_cached
    bacc._act_tables_patched = True


def _build_nc():
    _patch_act_tables()
    nc = bacc.Bacc()

    WDT = F8 if GATES == "fp8" else BF

    x_d = nc.dram_tensor("x", [C, HW], BF, kind="ExternalInput")
    wmc_d = nc.dram_tensor("wmc", [128, 64, 8], BF, kind="ExternalInput")
    bfc_d = nc.dram_tensor("bfc", [128, 1312], BF, kind="ExternalInput")
    sm_d = nc.dram_tensor("smalls", [128, 272], F32, kind="ExternalInput")
    if W1F8:
        wg1_d = nc.dram_tensor("wg1", [2, 4, 128, 2, 2048], F8,
                               kind="ExternalInput")
    else:
        wg1_d = nc.dram_tensor("wg1", [2, 8, 128, 2048], BF,
                               kind="ExternalInput")
    if W2F8:
        wg2_d = nc.dram_tensor("wg2", [2, 128, 4, 2, 512], F8,
                               kind="ExternalInput")
    else:
        wg2_d = nc.dram_tensor("wg2", [2, 128, 4096], BF,
                               kind="ExternalInput")
    out_d = nc.dram_tensor("out", [CL, HW], F32, kind="ExternalOutput")

    with tile.TileContext(nc) as tc, ExitStack() as ctx:
        resid = ctx.enter_context(tc.tile_pool(name="resid", bufs=1))
        spool = ctx.enter_context(tc.tile_pool(name="spool", bufs=1))
        stpool = ctx.enter_context(tc.tile_pool(name="stage", bufs=2))
        dpool = ctx.enter_context(tc.tile_pool(name="diag", bufs=1))

        x_sb = resid.tile([128, NJ, HW], BF, tag="x")
        wmc = resid.tile([128, 64, 8], BF, tag="wmc")
        bfc = resid.tile([128, 1312], BF, tag="bfc")
        sm = resid.tile([128, 272], F32, tag="sm")
        if W1F8:
            wg1_sb = resid.tile([128, 2, 4, 2, 2048], F8, tag="wg1")
        else:
            wg1_sb = resid.tile([128, 2, 8, 2048], BF, tag="wg1")
        if W2F8:
            wg2_sb = resid.tile([128, 2, 4, 2, 512], F8, tag="wg2")
        else:
            wg2_sb = resid.tile([128, 2, 4096], BF, tag="wg2")
        attn_bc = resid.tile([128, HW], BF, tag="attn_bc")
        scr = resid.tile([128, HW // 2], BF, tag="scr")
        if PASS2 == "half":
            # transposed copy of slabs 0-3: xT4[q, j, g, c] = x[128j+c, 128g+q]
            xT4 = resid.tile([128, 4, 32, 128], BF, tag="xT4")

        # DMA order on the sync queue: wmc, x (slab0 split), bfc/sm, weights
        nc.sync.dma_start(wmc[:], wmc_d[:])
        nc.sync.dma_start(x_sb[:, 0, 0:2048], x_d[0:128, 0:2048])
        nc.sync.dma_start(x_sb[:, 0, 2048:HW], x_d[0:128, 2048:HW])
        for j in range(1, NJ - 1):
            nc.sync.dma_start(x_sb[:, j, :], x_d[128 * j:128 * (j + 1), :])
        nc.sync.dma_start(x_sb[:, NJ - 1, 0:2048],
                          x_d[128 * (NJ - 1):C, 0:2048])
        nc.sync.dma_start(x_sb[:, NJ - 1, 2048:HW],
                          x_d[128 * (NJ - 1):C, 2048:HW])
        nc.sync.dma_start(bfc[:], bfc_d[:])
        nc.sync.dma_start(sm[:], sm_d[:])
        for r in range(R):
            if W1F8:
                for jp in range(4):
                    nc.sync.dma_start(wg1_sb[:, r, jp], wg1_d[r, jp])
            else:
                for j in range(NJ):
                    nc.sync.dma_start(wg1_sb[:, r, j, :], wg1_d[r, j])
            nc.sync.dma_start(wg2_sb[:, r], wg2_d[r])

        ident = bfc[:, 0:128]
        ident8 = bfc[0:8, 0:8]
        ones_col_bf = bfc[:, 128:129]
        ones8_bf = bfc[0:8, 128:129]
        one0 = bfc[0:1, 128:129]

        def eg(g):
            return bfc[0:8, 288 + 128 * g:288 + 128 * (g + 1)]

        onesf = sm[0:1, 128:256]

        def b1c(r):
            return sm[:, 16 * r:16 * r + 16]

        def gc(r):
            return sm[:, 32 + 16 * r:48 + 16 * r]

        def bec(r):
            return sm[:, 64 + 16 * r:80 + 16 * r]

        def b2c(r):
            return sm[:, 96 + 16 * r:112 + 16 * r]

        c256 = sm[:, 256:257]      # 1/256 column (fp8 W2 descale)
        cm256 = sm[0:1, 257:258]   # -1/256 (LN variance fold)

        # =============== phase A: logits (rides the x DMA) ===============
        attn8 = spool.tile([8, 512], BF, tag="attn8")
        acc8 = spool.tile([8, 1], F32, tag="acc8")
        inv = spool.tile([1, 1], F32, tag="inv")
        inv_bc = spool.tile([128, 1], F32, tag="inv_bc")
        v0 = spool.tile([128, NJ], F32, tag="v0")
        if W1F8:
            # [jp, t, pad16]: dual-fp8 LDWEIGHTS needs 16B-aligned k-tile step
            v0g = spool.tile([128, 4, 2, 16], F8, tag="v0g")
        else:
            v0g = spool.tile([128, NJ], BF, tag="v0g")

        with tc.tile_pool(name="psA", bufs=1,
                          space=bass.MemorySpace.PSUM) as psA:
            lg8 = psA.tile([8, 512], F32, tag="lg8")
            for j in range(NJ):
                for g in range(8):
                    nc.tensor.matmul(
                        lg8[:],
                        wmc[:, 8 * j + g, :],
                        x_sb[:, j, 512 * g:512 * (g + 1)],
                        start=(j == 0 and g == 0),
                        stop=(j == NJ - 1 and g == 7),
                    )
            # |logits| < ~4: softmax without max subtraction
            nc.scalar.activation(attn8[:], lg8[:], AF.Exp, accum_out=acc8[:])

            acc8b = spool.tile([8, 1], BF, tag="acc8b")
            nc.vector.tensor_copy(acc8b[:], acc8[:])
            ps_s = psA.tile([1, 1], F32, tag="small", bufs=1)
            nc.tensor.matmul(ps_s[:], acc8b[:], ones8_bf)
            nc.vector.reciprocal(inv[:], ps_s[:])
            if W1F8:
                nc.vector.tensor_scalar_mul(inv[:], inv[:], 64.0)
            ps_ib = psA.tile([128, 1], F32, tag="small", bufs=1)
            nc.tensor.matmul(ps_ib[:], onesf, inv[:])
            nc.vector.tensor_copy(inv_bc[:], ps_ib[:])
            scbf = spool.tile([128, 1], BF, tag="invbcb")
            nc.vector.tensor_copy(scbf[:], inv_bc[:])

            # attn broadcast (for the DVE half)
            for g in range(8):
                pb = psA.tile([128, 512], F32, tag="bcb",
                              name=f"bc{g % 2}")
                nc.tensor.matmul(pb[:], eg(g), attn8[:])
                dst = attn_bc[:, 512 * g:512 * (g + 1)]
                if g < 4:
                    nc.vector.tensor_copy(dst, pb[:])
                else:
                    nc.scalar.copy(dst, pb[:])

            if PASS2 == "half":
                # attn columns: attnT[p, k, g] = attn[512g + 128k + p]
                psAT = psA.tile([128, 4, 8], BF, tag="small", bufs=1)
                for k in range(4):
                    nc.tensor.transpose(
                        psAT[:, k, :], attn8[0:8, 128 * k:128 * (k + 1)],
                        ident8,
                    )
                attnT = spool.tile([128, 4, 8], BF, tag="attnT")
                nc.vector.tensor_copy(attnT[:], psAT[:])

                # transpose slabs 0-3 on the (now ramped) PE, overlapping
                # the DVE stt half
                for j in range(4):
                    for t in range(4):
                        px = psA.tile([128, 1024], BF, tag="xp",
                                      name=f"xp{t % 3}", bufs=3)
                        for u in range(8):
                            g8 = 8 * t + u
                            nc.tensor.transpose(
                                px[:, 128 * u:128 * (u + 1)],
                                x_sb[:, j, 128 * g8:128 * (g8 + 1)],
                                ident,
                            )
                        nc.scalar.copy(
                            xT4[:, j, 8 * t:8 * (t + 1), :],
                            px[:].rearrange("p (u c) -> p u c", c=128),
                        )

                # PE half of pass 2: channels 0:512 (slabs 0-3)
                ctx_ps = psA.tile([1, 512], F32, tag="ctx")
                for m in range(32):
                    nc.tensor.matmul(
                        ctx_ps[:],
                        attnT[:, m % 4, m // 4:m // 4 + 1],
                        xT4[:, :, m, :],
                        start=(m == 0), stop=(m == 31),
                    )
                v0row = spool.tile([1, 512], BF, tag="v0row")
                nc.vector.tensor_copy(v0row[:], ctx_ps[:])
                psV = psA.tile([128, 4], F32, tag="small", bufs=1)
                for q in range(4):
                    nc.tensor.matmul(
                        psV[:, q:q + 1],
                        v0row[0:1, 128 * q:128 * (q + 1)],
                        one0,
                    )
                nc.vector.tensor_scalar_mul(v0[:, 0:4], psV[:], inv_bc[:])

            # DVE half of pass 2 (slabs 4-7; all slabs in stt_bf mode)
            j0 = 4 if PASS2 == "half" else 0
            v0p = spool.tile([128, NJ, 2], F32, tag="v0p")
            for hh in range(2):
                for j in range(j0, NJ):
                    nc.vector.scalar_tensor_tensor(
                        out=scr[:],
                        in0=x_sb[:, j, 2048 * hh:2048 * (hh + 1)],
                        scalar=scbf[:],
                        in1=attn_bc[:, 2048 * hh:2048 * (hh + 1)],
                        op0=ALU.mult,
                        op1=ALU.mult,
                        accum_out=v0p[:, j, hh:hh + 1],
                    )
            nc.vector.reduce_sum(
                out=v0[:, j0:NJ],
                in_=v0p[:, j0:NJ, :],
                axis=AX.X,
            )

        if W1F8:
            nc.scalar.activation(
                v0g[:, :, :, 0],
                v0[:].rearrange("p (a b) -> p a b", b=2),
                AF.Identity,
            )
        else:
            nc.vector.tensor_copy(v0g[:], v0[:])

        # =============== gates ===============
        gates_ctx = tc.tile_pool(name="psg", bufs=1,
                                 space=bass.MemorySpace.PSUM)
        psg = gates_ctx.__enter__()

        def gate_round(r, vm_g, va_g, tag):
            # W1: h rows via v-stationary matmuls. psW order: m0 m1 a0 a1
            psW = [psg.tile([1, 512], F32, tag="w1p", name=f"w1p{k}", bufs=4)
                   for k in range(4)]
            if W1F8:
                for jp in range(4):
                    for p in range(2):
                        nc.tensor.matmul(
                            psW[p][:], vm_g[:, jp, :, 0:1],
                            wg1_sb[:, r, jp, :, 512 * p:512 * (p + 1)],
                            start=(jp == 0), stop=(jp == 3), perf_mode=DR,
                        )
                        nc.tensor.matmul(
                            psW[2 + p][:], va_g[:, jp, :, 0:1],
                            wg1_sb[:, r, jp, :,
                                   1024 + 512 * p:1024 + 512 * (p + 1)],
                            start=(jp == 0), stop=(jp == 3), perf_mode=DR,
                        )
            else:
                for j in range(NJ):
                    for p in range(2):
                        nc.tensor.matmul(
                            psW[2 + p][:], va_g[:, j:j + 1],
                            wg1_sb[:, r, j,
                                   1024 + 512 * p:1024 + 512 * (p + 1)],
                            start=(j == 0), stop=(j == NJ - 1),
                        )
                for j in range(NJ):
                    for p in range(2):
                        nc.tensor.matmul(
                            psW[p][:], vm_g[:, j:j + 1],
                            wg1_sb[:, r, j, 512 * p:512 * (p + 1)],
                            start=(j == 0), stop=(j == NJ - 1),
                        )
            hrow = spool.tile([1, 2048], BF, tag="rowbuf", name=f"hrow{tag}")
            for k in range(4):
                dst = hrow[0:1, 512 * k:512 * (k + 1)]
                if k % 2 == 0:
                    nc.vector.tensor_copy(dst, psW[k][:])
                else:
                    nc.scalar.copy(dst, psW[k][:])

            # transpose h rows -> [128,16] columns (b,l,t)
            psT = psg.tile([128, 16], F32, tag="tp", name=f"tp{tag}", bufs=2)
            for k in range(16):
                nc.tensor.matmul(
                    psT[:, k:k + 1],
                    hrow[0:1, 128 * k:128 * (k + 1)],
                    one0,
                )

            # LayerNorm over planes (groups of 2 columns = 256 planes).
            # g is pre-scaled by sqrt(P) on host; invsigma_noP =
            # exp(-0.5*ln(S2 - S1^2/P + P*EPS)); mu folded as S1/P.
            stcat = spool.tile([128, 32], BF, tag=f"stcat{tag}")
            nc.vector.tensor_add(stcat[:, 0:16], psT[:], b1c(r))
            nc.vector.tensor_mul(stcat[:, 16:32], stcat[:, 0:16],
                                 stcat[:, 0:16])
            ps_st = psg.tile([1, 32], F32, tag="tiny", bufs=2)
            nc.tensor.matmul(ps_st[:], ones_col_bf, stcat[:])

            w8 = spool.tile([1, 32], F32, tag=f"w8{tag}")
            nc.vector.reduce_sum(
                out=w8[0:1, 0:16],
                in_=ps_st[0:1, 0:32].rearrange("p (g t) -> p g t", t=2),
                axis=AX.X,
            )
            nc.vector.tensor_mul(w8[0:1, 16:24], w8[0:1, 0:8], w8[0:1, 0:8])
            nc.vector.scalar_tensor_tensor(
                out=w8[0:1, 8:16], in0=w8[0:1, 16:24], scalar=cm256,
                in1=w8[0:1, 8:16], op0=ALU.mult, op1=ALU.add,
            )
            nc.vector.tensor_scalar_add(w8[0:1, 8:16], w8[0:1, 8:16],
                                        P * EPS * KR[r] * KR[r])
            nc.scalar.activation(w8[0:1, 16:24], w8[0:1, 8:16], AF.Ln)
            nc.scalar.activation(w8[0:1, 24:32], w8[0:1, 16:24], AF.Exp,
                                 scale=-0.5)

            brow = spool.tile([1, 32], F32, tag=f"brow{tag}")
            bview = brow[0:1, 0:16].rearrange("p (g t) -> p t g", t=2)
            iview = brow[0:1, 16:32].rearrange("p (g t) -> p t g", t=2)
            for t in range(2):
                nc.vector.tensor_scalar_mul(bview[:, t, :], w8[0:1, 0:8],
                                            1.0 / P)
                nc.vector.tensor_copy(iview[:, t, :], w8[0:1, 24:32])
            ps_bc2 = psg.tile([128, 32], F32, tag="tp", name=f"tpb{tag}",
                              bufs=2)
            nc.tensor.matmul(ps_bc2[:], onesf, brow[:])

            hn = spool.tile([128, 16], F32, tag=f"hn{tag}")
            nc.vector.tensor_sub(hn[:], stcat[:, 0:16], ps_bc2[:, 0:16])
            nc.vector.tensor_mul(hn[:], hn[:], ps_bc2[:, 16:32])
            nc.vector.tensor_mul(hn[:], hn[:], gc(r))
            nc.vector.tensor_add(hn[:], hn[:], bec(r))
            if W2F8:
                # [b, l, t, pad16] for 16B-aligned dual-fp8 k-tile step
                hn_g = spool.tile([128, 2, 4, 2, 16], F8, tag=f"hnbf{tag}")
                nc.scalar.activation(
                    hn_g[:, :, :, :, 0],
                    hn[:].rearrange("p (b l t) -> p b l t", b=2, t=2),
                    AF.Relu, scale=16.0,
                )
            else:
                hn_g = spool.tile([128, 16], BF, tag=f"hnbf{tag}")
                nc.scalar.activation(hn_g[:], hn[:], AF.Relu)

            # W2: z rows via hn-stationary matmuls. psZ: m01 m23 a01 a23
            psZ = [psg.tile([1, 512], F32, tag="w1p", name=f"w1p{k}", bufs=4)
                   for k in range(4)]
            if W2F8:
                for lv in range(4):
                    for bb in range(2):
                        nc.tensor.matmul(
                            psZ[2 * bb + lv // 2][
                                0:1, 256 * (lv % 2):256 * (lv % 2) + 256],
                            hn_g[:, bb, lv, :, 0:1],
                            wg2_sb[:, r, lv, :, 256 * bb:256 * bb + 256],
                            perf_mode=DR,
                        )
            else:
                for lv in range(4):
                    for t in range(2):
                        nc.tensor.matmul(
                            psZ[lv // 2][0:1,
                                         256 * (lv % 2):256 * (lv % 2) + 256],
                            hn_g[:, 2 * lv + t:2 * lv + t + 1],
                            wg2_sb[:, r, 1024 * lv + 512 * t:
                                   1024 * lv + 512 * t + 256],
                            start=(t == 0), stop=(t == 1),
                        )
                        nc.tensor.matmul(
                            psZ[2 + lv // 2][
                                0:1, 256 * (lv % 2):256 * (lv % 2) + 256],
                            hn_g[:, 8 + 2 * lv + t:8 + 2 * lv + t + 1],
                            wg2_sb[:, r, 1024 * lv + 512 * t + 256:
                                   1024 * lv + 512 * t + 512],
                            start=(t == 0), stop=(t == 1),
                        )
            zrow = spool.tile([1, 2048], BF, tag="rowbuf", name=f"zrow{tag}")
            for k in range(4):
                dst = zrow[0:1, 512 * k:512 * (k + 1)]
                if k % 2 == 0:
                    nc.vector.tensor_copy(dst, psZ[k][:])
                else:
                    nc.scalar.copy(dst, psZ[k][:])
            psT2 = psg.tile([128, 16], F32, tag="tp", name=f"tp2{tag}", bufs=2)
            for k in range(16):
                nc.tensor.matmul(
                    psT2[:, k:k + 1],
                    zrow[0:1, 128 * k:128 * (k + 1)],
                    one0,
                )
            zcols = spool.tile([128, 16], F32, tag=f"zc{tag}")
            if W2F8:
                nc.vector.scalar_tensor_tensor(
                    out=zcols[:], in0=psT2[:], scalar=c256, in1=b2c(r),
                    op0=ALU.mult, op1=ALU.add,
                )
            else:
                nc.vector.tensor_add(zcols[:], psT2[:], b2c(r))
            return zcols

        zc0 = gate_round(0, v0g, v0g, "r0")

        # sigmoid(z) = 1/(1+exp(-z))
        def sigmoid(zview, tag):
            e = spool.tile([128, 8], F32, tag=f"sg{tag}")
            nc.scalar.activation(e[:], zview, AF.Exp, scale=-1.0)
            nc.vector.tensor_scalar_add(e[:], e[:], 1.0)
            s = spool.tile([128, 8], F32, tag=f"sg2{tag}")
            nc.vector.reciprocal(s[:], e[:])
            return s

        vmul = sigmoid(zc0[:, 0:8], "r0")
        vadd = zc0[:, 8:16]

        if W1F8:
            vm1 = spool.tile([128, 4, 2, 16], F8, tag="vm1")
            va1 = spool.tile([128, 4, 2, 16], F8, tag="va1")
            nc.scalar.activation(
                vm1[:, :, :, 0],
                vmul[:].rearrange("p (a b) -> p a b", b=2),
                AF.Identity, scale=16.0,
            )
            nc.scalar.activation(
                va1[:, :, :, 0],
                vadd.rearrange("p (a b) -> p a b", b=2),
                AF.Identity, scale=16.0,
            )
        else:
            vm1 = spool.tile([128, NJ], BF, tag="vm1")
            va1 = spool.tile([128, NJ], BF, tag="va1")
            nc.vector.tensor_copy(vm1[:], vmul[:])
            nc.vector.tensor_copy(va1[:], vadd)

        zc1 = gate_round(1, vm1, va1, "r1")
        s1 = sigmoid(zc1[:, 0:8], "r1")
        mm_f = spool.tile([128, NJ], F32, tag="mmf")
        nc.vector.tensor_add(mm_f[:], s1[:], vmul[:])
        ma_f = spool.tile([128, NJ], F32, tag="maf")
        nc.vector.tensor_add(ma_f[:], zc1[:, 8:16], vadd)
        gates_ctx.__exit__(None, None, None)

        # =============== pass 3: output ===============
        addsum = spool.tile([128, 2], F32, tag="addsum")
        nc.vector.reduce_sum(
            out=addsum[:],
            in_=ma_f[:].rearrange("p (l t) -> p t l", t=2),
            axis=AX.X,
        )
        diags = []
        for js in range(NJ):
            dt_ = dpool.tile([128, 128], BF, tag=f"diag{js}", name=f"diag{js}")
            nc.vector.tensor_scalar_mul(dt_[:], ident, mm_f[:, js:js + 1])
            diags.append(dt_)

        with tc.tile_pool(name="ps3", bufs=6,
                          space=bass.MemorySpace.PSUM) as ps3:
            for jj in range(2):
                for nch in range(NJ):
                    ps_o = ps3.tile([128, 512], F32, tag="big")
                    for lv in range(4):
                        js = 2 * lv + jj
                        nc.tensor.matmul(
                            ps_o[:],
                            diags[js][:],
                            x_sb[:, js, 512 * nch:512 * (nch + 1)],
                            start=(lv == 0), stop=(lv == 3),
                        )
                    stg = stpool.tile([128, 512], F32, tag="stg")
                    if (jj * 8 + nch) % 2 == 0:
                        nc.vector.tensor_scalar_add(
                            stg[:], ps_o[:], addsum[:, jj:jj + 1]
                        )
                    else:
                        nc.scalar.activation(
                            stg[:], ps_o[:], AF.Identity,
                            bias=addsum[:, jj:jj + 1], scale=1.0,
                        )
                    nc.sync.dma_start(
                        out_d[128 * jj:128 * (jj + 1),
                              512 * nch:512 * (nch + 1)],
                        stg[:],
                    )

    nc.compile()
    return nc


def _pack_inputs(x0, x1, x2, x3, wm, bm,
                 add_W1, add_b1, add_g, add_be, add_W2, add_b2,
                 mul_W1, mul_b1, mul_g, mul_be, mul_W2, mul_b2):
    bf = ml_dtypes.bfloat16
    f8 = ml_dtypes.float8_e4m3
    f32 = np.float32

    # pass1 stationaries: [:, 8j+g, g] = wm slab j
    wmr = np.asarray(wm, f32).reshape(NJ, 128)
    wmc = np.zeros((128, 64, 8), f32)
    for j in range(NJ):
        for g in range(8):
            wmc[:, 8 * j + g, g] = wmr[j]
    wmc = wmc.astype(bf)

    # bf16 const block: identity | ones col | pad | eg selectors
    bfc = np.zeros((128, 1312), f32)
    bfc[:, 0:128] = np.eye(128)
    bfc[:, 128] = 1.0
    for g in range(8):
        bfc[g, 288 + 128 * g:288 + 128 * (g + 1)] = 1.0
    bfc = bfc.astype(bf)

    W1s = [[mul_W1[r], add_W1[r]] for r in range(R)]
    W2s = [[mul_W2[r], add_W2[r]] for r in range(R)]
    b1s = [[mul_b1[r], add_b1[r]] for r in range(R)]
    gs = [[mul_g[r], add_g[r]] for r in range(R)]
    bes = [[mul_be[r], add_be[r]] for r in range(R)]
    b2s = [[mul_b2[r], add_b2[r]] for r in range(R)]

    sm = np.zeros((128, 272), f32)
    sm[:, 128:256] = 1.0
    sm[:, 256] = 1.0 / 256.0
    sm[:, 257] = -1.0 / 256.0

    def colmajor(v):  # [4,256]-like -> [128, 8] cols (l, t)
        return np.asarray(v, f32).reshape(4, 2, 128).transpose(2, 0, 1).reshape(128, 8)

    SW1 = 16.0 if W1F8 else 1.0
    SW2 = 16.0 if W2F8 else 1.0
    if W1F8:
        wg1 = np.zeros((2, 4, 128, 2, 2048), f32)
    else:
        wg1 = np.zeros((2, 8, 128, 2048), f32)
    if W2F8:
        wg2a = np.zeros((2, 128, 4, 2, 512), f32)
    else:
        wg2 = np.zeros((2, 128, 4096), f32)

    for r in range(R):
        if not W2F8:
            w2arr = np.zeros((128, 4, 2, 2, 256), f32)   # [pp, l, t, b, cl]
        for b in range(2):
            w1 = np.asarray(W1s[r][b], f32).reshape(C, C) * SW1  # [lp, c]
            t = w1.reshape(C, NJ, 128)                   # [q, j, cp]
            t = t.transpose(1, 2, 0)                     # [j, cp, q]
            if W1F8:
                # wg1[r, jp, cp, tt, 1024b + q] = w1[q, 128(2jp+tt)+cp]
                wg1[r, :, :, :, 1024 * b:1024 * (b + 1)] = \
                    t.reshape(4, 2, 128, C).transpose(0, 2, 1, 3)
            else:
                wg1[r, :, :, 1024 * b:1024 * (b + 1)] = t
            w2 = np.asarray(W2s[r][b], f32) * SW2        # [l, cl, pp]
            t2 = w2.reshape(4, 256, 2, 128)              # [l, cl, tt, pp]
            t2 = t2.transpose(3, 0, 2, 1)                # [pp, l, tt, cl]
            if W2F8:
                wg2a[r, :, :, :, 256 * b:256 * (b + 1)] = t2
            else:
                w2arr[:, :, :, b, :] = t2
            # h is scaled by KR[r] under fp8: scale b1 to match (LN absorbs)
            sm[:, 16 * r + 8 * b:16 * r + 8 * b + 8] = \
                colmajor(b1s[r][b]) * KR[r]
            sm[:, 32 + 16 * r + 8 * b:32 + 16 * r + 8 * b + 8] = \
                colmajor(gs[r][b]) * float(np.sqrt(P))
            sm[:, 64 + 16 * r + 8 * b:64 + 16 * r + 8 * b + 8] = \
                colmajor(bes[r][b])
            sm[:, 96 + 16 * r + 8 * b:96 + 16 * r + 8 * b + 8] = \
                colmajor(b2s[r][b])
        if not W2F8:
            wg2[r] = w2arr.reshape(128, 4096)

    shared = dict(wmc=wmc, bfc=bfc, smalls=sm,
                  wg1=wg1.astype(f8 if W1F8 else bf),
                  wg2=wg2a.astype(f8) if W2F8 else wg2.astype(bf))

    in_maps = []
    xs = [np.asarray(a, f32) for a in (x0, x1, x2, x3)]
    for b in range(B):
        xc = np.concatenate(
            [a[b].reshape(CL, HW) for a in xs], axis=0
        ).astype(bf)
        in_maps.append({"x": xc, **shared})
    return in_maps


def kernel(**inputs):
    from concourse.bass_utils import run_bass_kernel_spmd

    if "nc" not in _CACHE:
        _CACHE["nc"] = _build_nc()
    nc = _CACHE["nc"]

    in_maps = _pack_inputs(**inputs)
    res = run_bass_kernel_spmd(nc, in_maps, list(range(NCORES)))
    _CACHE["last_results"] = res
    out = np.stack(
        [res.results[b]["out"].reshape(CL, H, W) for b in range(B)]
    ).astype(np.float32)
    return out

